# revision 30
# baseline (speedup 1.0000x reference)
"""DSQG block (diagonal-sparse gated attention + FFN) on 8 NeuronCores.

v3 design:
- No xn AllGather: raw x is replicated to every core as fp8 (x*32 -> e4m3).
  Each core computes LN1 *stats* (mu, rstd) on its own 256-token f32 shard,
  AllGathers a tiny bf16 [2,256] stats row, and folds the normalization into
  the QKV epilogue: qkv[m,n] = rstd[n]*raw[m,n]/(sx*sw) - rstd[n]*mu[n]*csW[m] + b[m]
  with raw = (W*sw)^T (x*sx) accumulated via fp8 DoubleRow matmuls.
- Stats rows are broadcast across partitions with a full-rate [128,128]
  all-ones stationary matmul against a zeroed tile holding the row in
  partition 0 (K=1 broadcast matmuls measured 2.1ns/col; this is 0.42ns/col).
- Attention accumulates token-major (psO2[n, d|Z]) so the softmax divide is a
  per-partition tensor_scalar; the PE transpose back to feature-major is
  deferred by two iterations so the PE queue never stalls on the vector
  divide; the gate multiply is fused into the transpose evacuation.
- A2A of gated attention output is split into even/odd token tiles; chunk 0
  runs during the odd-tile half of attention. gfull unpack DMAs ride the
  gpsimd queue so they never block weight prefetch on the sync queue.
- LN2 sums are accumulated inline with the out-proj epilogue; rstd uses
  vector reciprocal + scalar Sqrt (one act table load instead of Ln+Exp).
- FFN runs as a single interleaved pipeline over the 32 hidden tiles: fc1
  produces hT[f], fc2 immediately accumulates it into a persistent
  [128, 8, 256] psum tile, so fc2 finishes with fc1 and its 8MB of weights
  stream across the whole FFN phase.
"""
import sys

sys.path.insert(0, "/opt/trn_rl_repo")

import numpy as np
import ml_dtypes

import concourse.bass as bass
import concourse.mybir as mybir
import concourse.tile as tile
from concourse import bacc
from concourse.bass_utils import run_bass_kernel_spmd
from concourse.masks import make_identity

BF16 = mybir.dt.bfloat16
F32 = mybir.dt.float32
F8 = mybir.dt.float8e4
AF = mybir.ActivationFunctionType
ALU = mybir.AluOpType
PERF2 = mybir.MatmulPerfMode.DoubleRow

N, D, H, HD, FF = 2048, 1024, 16, 64, 4096
NC = 8
NS = N // NC          # 256 sequence positions per core
NT = N // 128         # 16 global 128-row tiles
DT = D // 128         # 8 feature tiles
FT = FF // 128        # 32 hidden tiles
OFFSETS = tuple(sorted(set(range(0, 33)) | {48, 64, 96, 128, 192, 256, 384, 512, 768, 1024, 1536}))
DELTAS = [0, 1, 2, 3, 4, 6, 8, 12]   # block-diagonal offsets (x128)
EPS = 1e-5
SX = 32.0             # x fp8 scale
SW = 2048.0           # qkv/gate weight fp8 scale
T_ORDER = list(range(NT))  # natural; core i owns tiles {i, i+8}


def build_program():
    nc = bacc.Bacc("TRN2", target_bir_lowering=False, debug=False, num_devices=NC)

    xq8_d = nc.declare_dram_parameter("xq8", [128, DT, N], F8, isOutput=False)
    xTs_d = nc.declare_dram_parameter("xTs", [128, DT, NS], F32, isOutput=False)
    w8_d = nc.declare_dram_parameter("w8", [128, DT, 512], F8, isOutput=False)
    csw_d = nc.declare_dram_parameter("csw", [128, 4], F32, isOutput=False)
    bias4_d = nc.declare_dram_parameter("bias4", [128, 4], F32, isOutput=False)
    ow_d = nc.declare_dram_parameter("out_w", [128, DT, DT, 128], BF16, isOutput=False)
    ob_d = nc.declare_dram_parameter("out_b", [128, DT], F32, isOutput=False)
    w1_d = nc.declare_dram_parameter("fc1_w", [128, FT, DT, 128], BF16, isOutput=False)
    b1_d = nc.declare_dram_parameter("fc1_b", [128, FT], F32, isOutput=False)
    w2_d = nc.declare_dram_parameter("fc2_w", [128, FT, DT, 128], BF16, isOutput=False)
    b2_d = nc.declare_dram_parameter("fc2_b", [128, DT], F32, isOutput=False)
    mk_d = nc.declare_dram_parameter("masks", [128, 2, 8, 128], BF16, isOutput=False)
    y_d = nc.declare_dram_parameter("yT", [128, DT, NS], F32, isOutput=True)

    with tile.TileContext(nc) as tc:
        with (
            tc.tile_pool(name="consts", bufs=1) as consts,
            tc.tile_pool(name="state", bufs=1) as state,
            tc.tile_pool(name="sq", bufs=3) as sqp,
            tc.tile_pool(name="small", bufs=2) as small,
            tc.tile_pool(name="epool", bufs=3) as epool,
            tc.tile_pool(name="zpool", bufs=4) as zpool,
            tc.tile_pool(name="osb", bufs=4) as osbp,
            tc.tile_pool(name="dram", bufs=1, space="DRAM") as dram,
        ):
            # ---------- loads: xq8/w8 first (stats + QKV), then the rest ----------
            w8 = consts.tile([128, DT, 512], F8)
            for dt in range(0, DT, 2):
                nc.sync.dma_start(out=w8[:, dt:dt + 2, :], in_=w8_d.ap()[:, dt:dt + 2, :])
            xq8t = consts.tile([128, DT, N], F8)
            for dt in range(DT):
                nc.sync.dma_start(out=xq8t[:, dt, :], in_=xq8_d.ap()[:, dt, :])
            csw = consts.tile([128, 4], F32)
            nc.sync.dma_start(out=csw[:], in_=csw_d.ap())
            bias4 = consts.tile([128, 4], F32)
            nc.sync.dma_start(out=bias4[:], in_=bias4_d.ap())
            xTs = consts.tile([128, DT, NS], F32)
            for dt in range(DT):
                nc.sync.dma_start(out=xTs[:, dt, :], in_=xTs_d.ap()[:, dt, :])
            mk = consts.tile([128, 2, 8, 128], BF16)
            nc.sync.dma_start(out=mk[:], in_=mk_d.ap())
            ob = consts.tile([128, DT], F32)
            nc.sync.dma_start(out=ob[:], in_=ob_d.ap())
            b1t = consts.tile([128, FT], F32)
            nc.sync.dma_start(out=b1t[:], in_=b1_d.ap())
            b2t = consts.tile([128, DT], F32)
            nc.sync.dma_start(out=b2t[:], in_=b2_d.ap())
            w2ball = consts.tile([128, FT, 2, 128], BF16)
            nc.sync.dma_start(out=w2ball[:], in_=w2_d.ap()[:, :, 6:8, :])

            ident = consts.tile([128, 128], BF16)
            make_identity(nc, ident[:])
            ones_c = consts.tile([128, 1], F32)
            nc.vector.memset(ones_c[:], 1.0)
            ones_r = consts.tile([1, 128], F32)
            nc.vector.memset(ones_r[:], 1.0)
            ones128 = consts.tile([128, 128], BF16)
            nc.vector.memset(ones128[:], 1.0)
            ones8 = consts.tile([128, 128], F8)
            nc.vector.memset(ones8[:], 1.0)

            # ---------- state ----------
            qT = state.tile([128, N], BF16)
            kT = state.tile([128, N], BF16)
            vT = state.tile([128, N], BF16)
            gateT = state.tile([128, N], BF16)
            gatedT = state.tile([128, N], BF16)
            vaug = state.tile([128, NT, 130], BF16)
            gfull = state.tile([128, DT, NS], BF16)
            x2T = state.tile([128, DT, NS], F32)
            xn2T = state.tile([128, DT, NS], BF16)
            hT = state.tile([128, FT, NS], BF16, tag="bigshare")
            qkctx = tc.tile_pool(name="qkst", bufs=1)
            qkst = qkctx.__enter__()
            R_bc = qkst.tile([128, N], F32, tag="R_bc")
            G_bc = qkst.tile([128, 4, N], BF16, tag="G_bc")

            # ---------- full-N LN1 stats from replicated fp8 x (no collective) ----------
            with tc.tile_pool(name="stps", bufs=2, space="PSUM") as stps:
                for c in range(4):
                    cols = slice(512 * c, 512 * c + 512)
                    psmu = stps.tile([128, 512], F32, tag="psmu")
                    for dt in range(DT):
                        nc.tensor.matmul(psmu[:], ones8[:], xq8t[:, dt, cols],
                                         start=(dt == 0), stop=(dt == DT - 1))
                    pssq = stps.tile([128, 512], F32, tag="pssq")
                    for dt in range(DT):
                        sq8 = sqp.tile([128, 512], BF16, tag="sq8")
                        nc.vector.tensor_tensor(out=sq8[:], in0=xq8t[:, dt, cols],
                                                in1=xq8t[:, dt, cols], op=ALU.mult)
                        nc.tensor.matmul(pssq[:], ones128[:], sq8[:],
                                         start=(dt == 0), stop=(dt == DT - 1))
                    mu_sb = sqp.tile([128, 512], F32, tag="musb")
                    nc.vector.tensor_copy(out=mu_sb[:], in_=psmu[:])
                    t_v = sqp.tile([128, 512], F32, tag="evac")
                    nc.vector.tensor_scalar(out=t_v[:], in0=pssq[:],
                                            scalar1=float(1.0 / (D * SX * SX)), scalar2=float(EPS),
                                            op0=ALU.mult, op1=ALU.add)
                    m2_v = sqp.tile([128, 512], F32, tag="evac2")
                    nc.vector.tensor_tensor(out=m2_v[:], in0=mu_sb[:], in1=mu_sb[:], op=ALU.mult)
                    var_v = sqp.tile([128, 512], F32, tag="varc")
                    nc.vector.scalar_tensor_tensor(out=var_v[:], in0=m2_v[:],
                                                   scalar=float(-1.0 / (D * SX) ** 2), in1=t_v[:],
                                                   op0=ALU.mult, op1=ALU.add)
                    rvar_v = sqp.tile([128, 512], F32, tag="rvarc")
                    nc.vector.reciprocal(out=rvar_v[:], in_=var_v[:])
                    nc.scalar.activation(out=R_bc[:, cols], in_=rvar_v[:], func=AF.Sqrt,
                                         scale=float(1.0 / (SX * SW) ** 2))
                    mrc = sqp.tile([128, 512], F32, tag="mrc")
                    nc.vector.scalar_tensor_tensor(out=mrc[:], in0=mu_sb[:],
                                                   scalar=float(-SW / D), in1=R_bc[:, cols],
                                                   op0=ALU.mult, op1=ALU.mult)
                    for g in range(4):
                        nc.vector.tensor_scalar(
                            out=G_bc[:, g, cols], in0=mrc[:],
                            scalar1=csw[:, g:g + 1], scalar2=bias4[:, g:g + 1],
                            op0=ALU.mult, op1=ALU.add,
                        )

            # ---------- QKV+gate fp8 DoubleRow matmuls over raw x ----------
            with tc.tile_pool(name="qkps", bufs=6, space="PSUM") as qkps:
                ps_list = []
                for g in range(4):
                    for c2 in range(4):
                        ps = qkps.tile([128, 512], F32)
                        for kp in range(4):
                            nc.tensor.matmul(
                                ps[:],
                                w8[:, 2 * kp:2 * kp + 2, 128 * g:128 * g + 128],
                                xq8t[:, 2 * kp:2 * kp + 2, 512 * c2:512 * c2 + 512],
                                start=(kp == 0), stop=(kp == 3),
                                perf_mode=PERF2,
                            )
                        ps_list.append((g, c2, ps))

                if True:
                    # epilogue: dequant + LN fold + bias (+ sigmoid for gate)
                    for g, c2, ps in ps_list:
                        cols = slice(512 * c2, 512 * c2 + 512)
                        t1 = sqp.tile([128, 512], F32, tag="evac")
                        nc.vector.tensor_tensor(out=t1[:], in0=ps[:], in1=R_bc[:, cols], op=ALU.mult)
                        if g < 3:
                            dst = (qT, kT, vT)[g]
                            eng = nc.gpsimd if g in (1, 2) else nc.vector
                            eng.tensor_tensor(out=dst[:, cols], in0=t1[:], in1=G_bc[:, g, cols], op=ALU.add)
                        else:
                            t2 = sqp.tile([128, 512], F32, tag="evac2")
                            nc.vector.tensor_tensor(out=t2[:], in0=t1[:], in1=G_bc[:, g, cols], op=ALU.add)
                            nc.scalar.activation(out=gateT[:, cols], in_=t2[:], func=AF.Sigmoid)

            qkctx.__exit__(None, None, None)

            # ---------- v rows (PE transpose) + ones column ----------
            with tc.tile_pool(name="trps", bufs=2, space="PSUM") as trps:
                for b in range(NT):
                    pst = trps.tile([128, 128], BF16)
                    nc.tensor.transpose(pst[:], vT[:, 128 * b:128 * b + 128], ident[:])
                    nc.scalar.activation(out=vaug[:, b, 0:64], in_=pst[:, 0:64], func=AF.Copy)
                    nc.scalar.activation(out=vaug[:, b, 65:129], in_=pst[:, 64:128], func=AF.Copy)
            nc.vector.memset(vaug[:, :, 64:65], 1.0)
            nc.vector.memset(vaug[:, :, 129:130], 1.0)

            # ---------- attention (token-major, lag-2 deferred transpose) ----------
            a2a_in0 = dram.tile([NC, 128, 128], BF16, tag="a2ain0")
            a2a_in1 = dram.tile([NC, 128, 128], BF16, tag="a2ain1")
            a2a_out0 = dram.tile([NC, 128, 128], BF16, tag="a2aout0")
            a2a_out1 = dram.tile([NC, 128, 128], BF16, tag="a2aout1")
            a2a_in = [a2a_in0, a2a_in1]
            a2a_out = [a2a_out0, a2a_out1]
            with (
                tc.tile_pool(name="scps", bufs=2, space="PSUM") as scps,
                tc.tile_pool(name="ops", bufs=2, space="PSUM") as ops,
                tc.tile_pool(name="tps", bufs=2, space="PSUM") as tps,
            ):
                pending = []

                def emit_tail(t, hp, o_sb):
                    psT = tps.tile([64, 128], BF16)
                    nc.tensor.transpose(psT[:], o_sb[:], ident[:])
                    nc.vector.tensor_tensor(
                        out=gatedT[64 * hp:64 * hp + 64, 128 * t:128 * t + 128],
                        in0=psT[:], in1=gateT[64 * hp:64 * hp + 64, 128 * t:128 * t + 128],
                        op=ALU.mult)

                for ti, t in enumerate(T_ORDER):
                    for hp in range(2):
                        p_t = sum(1 for dl in DELTAS if dl <= t)
                        psS = scps.tile([128, 1024], F32)
                        for s in range(p_t):
                            b = t - DELTAS[s]
                            nc.tensor.matmul(psS[:, 128 * s:128 * s + 128],
                                             kT[64 * hp:64 * hp + 64, 128 * b:128 * b + 128],
                                             qT[64 * hp:64 * hp + 64, 128 * t:128 * t + 128],
                                             start=True, stop=True)
                        E = epool.tile([128, 1024], BF16)
                        nc.scalar.activation(out=E[:, :128 * p_t], in_=psS[:, :128 * p_t],
                                             func=AF.Exp, scale=float(HD ** -0.5))
                        meng = nc.gpsimd if hp == 1 else nc.vector
                        meng.tensor_tensor(out=E[:, :128 * p_t], in0=E[:, :128 * p_t],
                                           in1=mk[:, hp, 0:p_t, :], op=ALU.mult)
                        psO = ops.tile([128, 65], F32)
                        for s in range(p_t):
                            b = t - DELTAS[s]
                            nc.tensor.matmul(psO[:], E[:, 128 * s:128 * s + 128],
                                             vaug[:, b, 65 * hp:65 * hp + 65],
                                             start=(s == 0), stop=(s == p_t - 1))
                        zinv = zpool.tile([128, 1], F32)
                        nc.vector.reciprocal(out=zinv[:], in_=psO[:, 64:65])
                        o_sb = osbp.tile([128, 64], BF16)
                        nc.vector.tensor_scalar_mul(out=o_sb[:], in0=psO[:, 0:64], scalar1=zinv[:, 0:1])
                        pending.append((t, hp, o_sb))
                        while len(pending) > 2:
                            emit_tail(*pending.pop(0))
                    if ti == NT // 2 - 1 or ti == NT - 1:
                        while pending:
                            emit_tail(*pending.pop(0))
                        c = 0 if ti == NT // 2 - 1 else 1
                        nc.gpsimd.dma_start(
                            out=a2a_in[c][:].rearrange("s p n -> p s n"),
                            in_=gatedT[:].rearrange("p (s n) -> p s n", s=NT)[:, 8 * c:8 * c + 8, :])
                        nc.gpsimd.collective_compute(
                            "AllToAll", ALU.bypass,
                            replica_groups=[list(range(NC))],
                            ins=[a2a_in[c].opt()], outs=[a2a_out[c].opt()],
                        )
                        nc.gpsimd.dma_start(out=gfull[:, :, 128 * c:128 * c + 128],
                                            in_=a2a_out[c][:].rearrange("dt p n -> p dt n"))

            w1ctx = tc.tile_pool(name="w1p", bufs=6)
            w1p = w1ctx.__enter__()
            w2ctx = tc.tile_pool(name="w2p", bufs=3)
            w2p = w2ctx.__enter__()

            # ---------- out proj + residual in token halves, LN2 sums inlined ----------
            x2sum = sqp.tile([128, NS], F32, tag="x2sum")
            x2sq = sqp.tile([128, NS], F32, tag="x2sq")
            with tc.tile_pool(name="mps", bufs=2, space="PSUM") as mps:
                for half in range(2):
                    hc = slice(128 * half, 128 * half + 128)
                    for m in range(DT):
                        owt = w1p.tile([128, DT, 128], BF16, tag="wchunk")
                        nc.sync.dma_start(out=owt[:], in_=ow_d.ap()[:, m, :, :])
                        ps = mps.tile([128, 128], F32)
                        for kt in range(DT):
                            nc.tensor.matmul(ps[:], owt[:, kt, :], gfull[:, kt, hc],
                                             start=(kt == 0), stop=(kt == DT - 1))
                        nc.vector.scalar_tensor_tensor(out=x2T[:, m, hc], in0=ps[:],
                                                       scalar=ob[:, m:m + 1], in1=xTs[:, m, hc],
                                                       op0=ALU.add, op1=ALU.add)
                        sqm = sqp.tile([128, 128], F32, tag="lnsqt")
                        nc.vector.tensor_tensor(out=sqm[:], in0=x2T[:, m, hc], in1=x2T[:, m, hc], op=ALU.mult)
                        if m == 0:
                            nc.vector.tensor_copy(out=x2sum[:, hc], in_=x2T[:, m, hc])
                            nc.vector.tensor_copy(out=x2sq[:, hc], in_=sqm[:])
                        else:
                            nc.vector.tensor_tensor(out=x2sum[:, hc], in0=x2sum[:, hc], in1=x2T[:, m, hc], op=ALU.add)
                            nc.vector.tensor_tensor(out=x2sq[:, hc], in0=x2sq[:, hc], in1=sqm[:], op=ALU.add)

            # ---------- LN2 stats + apply ----------
            with tc.tile_pool(name="ln2ps", bufs=1, space="PSUM") as ln2ps:
                ps_mu2 = ln2ps.tile([1, NS], F32, tag="mu2")
                ps_sq2 = ln2ps.tile([1, NS], F32, tag="sq2")
                nc.tensor.matmul(ps_mu2[:], ones_c[:], x2sum[:], start=True, stop=True)
                nc.tensor.matmul(ps_sq2[:], ones_c[:], x2sq[:], start=True, stop=True)
                mean2 = small.tile([1, NS], F32)
                nc.vector.tensor_scalar_mul(out=mean2[:], in0=ps_mu2[:], scalar1=1.0 / D)
                ex22 = small.tile([1, NS], F32)
                nc.vector.tensor_scalar_mul(out=ex22[:], in0=ps_sq2[:], scalar1=1.0 / D)
            m22 = small.tile([1, NS], F32)
            nc.vector.tensor_tensor(out=m22[:], in0=mean2[:], in1=mean2[:], op=ALU.mult)
            var2 = small.tile([1, NS], F32)
            nc.vector.tensor_scalar(out=var2[:], in0=ex22[:], scalar1=float(EPS), scalar2=None,
                                    op0=ALU.add)
            nc.vector.tensor_tensor(out=var2[:], in0=var2[:], in1=m22[:], op=ALU.subtract)
            rvar2 = small.tile([1, NS], F32)
            nc.vector.reciprocal(out=rvar2[:], in_=var2[:])
            rstd2 = small.tile([1, NS], F32)
            nc.scalar.activation(out=rstd2[:], in_=rvar2[:], func=AF.Sqrt)
            with tc.tile_pool(name="lnbc", bufs=1, space="PSUM") as lnbc:
                ps_mbc = lnbc.tile([128, NS], F32, tag="mbc2")
                nc.tensor.matmul(ps_mbc[:], ones_r[:], mean2[:], start=True, stop=True)
                ps_rbc = lnbc.tile([128, NS], F32, tag="rbc2")
                nc.tensor.matmul(ps_rbc[:], ones_r[:], rstd2[:], start=True, stop=True)
                for dt in range(DT):
                    tmp_t = sqp.tile([128, NS], F32, tag="lntmp")
                    nc.vector.tensor_tensor(out=tmp_t[:], in0=x2T[:, dt, :],
                                            in1=ps_mbc[:], op=ALU.subtract)
                    nc.vector.tensor_tensor(out=xn2T[:, dt, :], in0=tmp_t[:],
                                            in1=ps_rbc[:], op=ALU.mult)

            # ---------- FFN: fc1 with 6 fc2 chains interleaved + 2-chain tail ----------
            # each fc2 accumulation chain owns a full PSUM bank (512-col f32
            # pitch): interleaved chains sharing a bank lose data on `start`.
            with tc.tile_pool(name="accps", bufs=1, space="PSUM") as accps:
                psA = accps.tile([128, 6, 512], F32, tag="psA")
                fc2_pend = []

                def emit_fc2A(f, w2t):
                    for m in range(6):
                        nc.tensor.matmul(psA[:, m, 0:NS], w2t[:, m, :], hT[:, f, :],
                                         start=(f == 0), stop=(f == FT - 1))

                with tc.tile_pool(name="f1ps", bufs=2, space="PSUM") as f1ps:
                    for f in range(FT):
                        w1t = w1p.tile([128, DT, 128], BF16, tag="wchunk")
                        nc.sync.dma_start(out=w1t[:], in_=w1_d.ap()[:, f, :, :])
                        ps = f1ps.tile([128, NS], F32)
                        for kt in range(DT):
                            nc.tensor.matmul(ps[:], w1t[:, kt, :], xn2T[:, kt, :],
                                             start=(kt == 0), stop=(kt == DT - 1))
                        nc.scalar.activation(out=hT[:, f, :], in_=ps[:], func=AF.Gelu,
                                             bias=b1t[:, f:f + 1])
                        w2t = w2p.tile([128, 6, 128], BF16, tag="w2a")
                        nc.sync.dma_start(out=w2t[:], in_=w2_d.ap()[:, f, 0:6, :])
                        fc2_pend.append((f, w2t))
                        if len(fc2_pend) > 1:
                            emit_fc2A(*fc2_pend.pop(0))
                    while fc2_pend:
                        emit_fc2A(*fc2_pend.pop(0))
                with tc.tile_pool(name="accpsB", bufs=1, space="PSUM") as accpsB:
                    psB = accpsB.tile([128, 2, 512], F32, tag="psB")
                    for f in range(FT):
                        for m in range(2):
                            nc.tensor.matmul(psB[:, m, 0:NS], w2ball[:, f, m, :], hT[:, f, :],
                                             start=(f == 0), stop=(f == FT - 1))
                    for m in range(DT):
                        src_ps = psA[:, m, 0:NS] if m < 6 else psB[:, m - 6, 0:NS]
                        yt = sqp.tile([128, NS], F32, tag="yout")
                        nc.vector.scalar_tensor_tensor(out=yt[:], in0=src_ps,
                                                       scalar=b2t[:, m:m + 1], in1=x2T[:, m, :],
                                                       op0=ALU.add, op1=ALU.add)
                        nc.sync.dma_start(out=y_d.ap()[:, m, :], in_=yt[:])
            w2ctx.__exit__(None, None, None)
            w1ctx.__exit__(None, None, None)

    nc.finalize()
    return nc


_BF = ml_dtypes.bfloat16
_F8 = mybir.dt.np(F8)


def _bf(a):
    return np.ascontiguousarray(np.asarray(a, dtype=np.float32).astype(_BF))


def _q8(a, scale):
    return np.ascontiguousarray(
        np.clip(np.asarray(a, np.float32) * scale, -240.0, 240.0).astype(_F8))


def _pmajor(w):
    """w (K, M) -> [128, M//128, K//128, 128] p-major contiguous."""
    K, M = w.shape
    return np.ascontiguousarray(
        w.reshape(K // 128, 128, M // 128, 128).transpose(1, 2, 0, 3))


def _ktmajor(w):
    """w (K, M) -> [128, K//128, M//128, 128] kt-major contiguous."""
    K, M = w.shape
    return np.ascontiguousarray(
        w.reshape(K // 128, 128, M // 128, 128).transpose(1, 0, 2, 3))


def _prep_inputs(inputs):
    x = np.asarray(inputs["x"], dtype=np.float32)[0]          # (N, D)
    g1 = np.asarray(inputs["ln1_g"], np.float32); b1 = np.asarray(inputs["ln1_b"], np.float32)
    g2 = np.asarray(inputs["ln2_g"], np.float32); b2 = np.asarray(inputs["ln2_b"], np.float32)
    qkv_w = np.asarray(inputs["qkv_w"], np.float32); qkv_b = np.asarray(inputs["qkv_b"], np.float32)
    gate_w = np.asarray(inputs["gate_w"], np.float32); gate_b = np.asarray(inputs["gate_b"], np.float32)
    out_w = np.asarray(inputs["out_w"], np.float32); out_b = np.asarray(inputs["out_b"], np.float32)
    fc1_w = np.asarray(inputs["fc1_w"], np.float32); fc1_b = np.asarray(inputs["fc1_b"], np.float32)
    fc2_w = np.asarray(inputs["fc2_w"], np.float32); fc2_b = np.asarray(inputs["fc2_b"], np.float32)
    pos_bias = np.asarray(inputs["pos_bias"], np.float32)     # (O, H)

    xT = np.ascontiguousarray(x.T)                            # (D, N)
    qkvw_eff = g1[:, None] * qkv_w
    qkvb_eff = qkv_b + b1 @ qkv_w
    gatew_eff = g1[:, None] * gate_w
    gateb_eff = gate_b + b1 @ gate_w
    fc1w_eff = g2[:, None] * fc1_w
    fc1b_eff = fc1_b + b2 @ fc1_w

    xq8 = np.ascontiguousarray(
        _q8(xT, SX).reshape(DT, 128, N).transpose(1, 0, 2))
    ow_p = _pmajor(_bf(out_w))
    w1_p = _pmajor(_bf(fc1w_eff))
    w2_p = _ktmajor(_bf(fc2_w))
    ob_pack = np.ascontiguousarray(out_b.reshape(DT, 128).T)
    b1_pack = np.ascontiguousarray(fc1b_eff.reshape(FT, 128).T)
    b2_pack = np.ascontiguousarray(fc2_b.reshape(DT, 128).T)

    offs = np.asarray(OFFSETS)
    in_maps = []
    for i in range(NC):
        qcols = slice(128 * i, 128 * i + 128)
        kcols = slice(D + 128 * i, D + 128 * i + 128)
        vcols = slice(2 * D + 128 * i, 2 * D + 128 * i + 128)
        W_all = np.concatenate([qkvw_eff[:, qcols], qkvw_eff[:, kcols],
                                qkvw_eff[:, vcols], gatew_eff[:, qcols]], axis=1)  # (D, 512)
        w8 = np.ascontiguousarray(
            _q8(W_all, SW).reshape(DT, 128, 512).transpose(1, 0, 2))
        csw = np.ascontiguousarray(W_all.sum(0).reshape(4, 128).T.astype(np.float32))
        bias4 = np.stack([qkvb_eff[qcols], qkvb_eff[kcols], qkvb_eff[vcols],
                          gateb_eff[qcols]], axis=1).astype(np.float32)
        masks = np.zeros((2, 8, 128, 128), np.float32)
        jj, nn = np.meshgrid(np.arange(128), np.arange(128), indexing="ij")
        for hp in range(2):
            h = 2 * i + hp
            for s, dl in enumerate(DELTAS):
                delta = 128 * dl + nn - jj
                valid = np.isin(delta, offs)
                pb = np.zeros((128, 128), np.float32)
                pb[valid] = pos_bias[np.searchsorted(offs, delta[valid]), h]
                masks[hp, s] = np.where(valid, np.exp(pb), 0.0)
        mk_p = np.ascontiguousarray(_bf(masks).transpose(2, 0, 1, 3))
        xcols = np.concatenate([xT[:, 128 * i:128 * i + 128],
                                xT[:, 128 * (i + 8):128 * (i + 8) + 128]], axis=1)
        xTs = np.ascontiguousarray(xcols.reshape(DT, 128, NS).transpose(1, 0, 2))
        in_maps.append({
            "xq8": xq8, "xTs": xTs,
            "w8": w8, "csw": csw, "bias4": bias4,
            "out_w": ow_p, "out_b": ob_pack,
            "fc1_w": w1_p, "fc1_b": b1_pack,
            "fc2_w": w2_p, "fc2_b": b2_pack,
            "masks": mk_p,
        })
    return in_maps


_PROGRAM = None


def _get_program():
    global _PROGRAM
    if _PROGRAM is None:
        _PROGRAM = build_program()
    return _PROGRAM


def run(inputs, **run_kwargs):
    nc = _get_program()
    in_maps = _prep_inputs(inputs)
    res = run_bass_kernel_spmd(nc, in_maps, core_ids=list(range(NC)), **run_kwargs)
    yT = np.zeros((D, N), np.float32)
    for i in range(NC):
        yp = res.results[i]["yT"].transpose(1, 0, 2).reshape(D, NS)  # (D, 256)
        yT[:, 128 * i:128 * i + 128] = yp[:, 0:128]
        yT[:, 128 * (i + 8):128 * (i + 8) + 128] = yp[:, 128:256]
    return np.ascontiguousarray(yT.T)[None], res


def kernel(**inputs):
    y, _ = run(inputs)
    return y


# revision 31
# speedup vs baseline: 1.0040x; 1.0040x over previous
"""DSQG block (diagonal-sparse gated attention + FFN) on 8 NeuronCores.

v3 design:
- No xn AllGather: raw x is replicated to every core as fp8 (x*32 -> e4m3).
  Each core computes LN1 *stats* (mu, rstd) on its own 256-token f32 shard,
  AllGathers a tiny bf16 [2,256] stats row, and folds the normalization into
  the QKV epilogue: qkv[m,n] = rstd[n]*raw[m,n]/(sx*sw) - rstd[n]*mu[n]*csW[m] + b[m]
  with raw = (W*sw)^T (x*sx) accumulated via fp8 DoubleRow matmuls.
- Stats rows are broadcast across partitions with a full-rate [128,128]
  all-ones stationary matmul against a zeroed tile holding the row in
  partition 0 (K=1 broadcast matmuls measured 2.1ns/col; this is 0.42ns/col).
- Attention accumulates token-major (psO2[n, d|Z]) so the softmax divide is a
  per-partition tensor_scalar; the PE transpose back to feature-major is
  deferred by two iterations so the PE queue never stalls on the vector
  divide; the gate multiply is fused into the transpose evacuation.
- A2A of gated attention output is split into even/odd token tiles; chunk 0
  runs during the odd-tile half of attention. gfull unpack DMAs ride the
  gpsimd queue so they never block weight prefetch on the sync queue.
- LN2 sums are accumulated inline with the out-proj epilogue; rstd uses
  vector reciprocal + scalar Sqrt (one act table load instead of Ln+Exp).
- FFN runs as a single interleaved pipeline over the 32 hidden tiles: fc1
  produces hT[f], fc2 immediately accumulates it into a persistent
  [128, 8, 256] psum tile, so fc2 finishes with fc1 and its 8MB of weights
  stream across the whole FFN phase.
"""
import sys

sys.path.insert(0, "/opt/trn_rl_repo")

import numpy as np
import ml_dtypes

import concourse.bass as bass
import concourse.mybir as mybir
import concourse.tile as tile
from concourse import bacc
from concourse.bass_utils import run_bass_kernel_spmd
from concourse.masks import make_identity

BF16 = mybir.dt.bfloat16
F32 = mybir.dt.float32
F8 = mybir.dt.float8e4
AF = mybir.ActivationFunctionType
ALU = mybir.AluOpType
PERF2 = mybir.MatmulPerfMode.DoubleRow

N, D, H, HD, FF = 2048, 1024, 16, 64, 4096
NC = 8
NS = N // NC          # 256 sequence positions per core
NT = N // 128         # 16 global 128-row tiles
DT = D // 128         # 8 feature tiles
FT = FF // 128        # 32 hidden tiles
OFFSETS = tuple(sorted(set(range(0, 33)) | {48, 64, 96, 128, 192, 256, 384, 512, 768, 1024, 1536}))
DELTAS = [0, 1, 2, 3, 4, 6, 8, 12]   # block-diagonal offsets (x128)
EPS = 1e-5
SX = 32.0             # x fp8 scale
SW = 2048.0           # qkv/gate weight fp8 scale
T_ORDER = list(range(NT))  # natural; core i owns tiles {i, i+8}


def build_program():
    nc = bacc.Bacc("TRN2", target_bir_lowering=False, debug=False, num_devices=NC)

    xq8_d = nc.declare_dram_parameter("xq8", [128, DT, N], F8, isOutput=False)
    xTs_d = nc.declare_dram_parameter("xTs", [128, DT, NS], F32, isOutput=False)
    w8_d = nc.declare_dram_parameter("w8", [128, DT, 512], F8, isOutput=False)
    csw_d = nc.declare_dram_parameter("csw", [128, 4], F32, isOutput=False)
    bias4_d = nc.declare_dram_parameter("bias4", [128, 4], F32, isOutput=False)
    ow_d = nc.declare_dram_parameter("out_w", [128, DT, DT, 128], BF16, isOutput=False)
    ob_d = nc.declare_dram_parameter("out_b", [128, DT], F32, isOutput=False)
    w1_d = nc.declare_dram_parameter("fc1_w", [128, FT, DT, 128], BF16, isOutput=False)
    b1_d = nc.declare_dram_parameter("fc1_b", [128, FT], F32, isOutput=False)
    w2_d = nc.declare_dram_parameter("fc2_w", [128, FT, DT, 128], BF16, isOutput=False)
    b2_d = nc.declare_dram_parameter("fc2_b", [128, DT], F32, isOutput=False)
    mk_d = nc.declare_dram_parameter("masks", [128, 2, 8, 128], BF16, isOutput=False)
    y_d = nc.declare_dram_parameter("yT", [128, DT, NS], F32, isOutput=True)

    with tile.TileContext(nc) as tc:
        with (
            tc.tile_pool(name="consts", bufs=1) as consts,
            tc.tile_pool(name="state", bufs=1) as state,
            tc.tile_pool(name="sq", bufs=3) as sqp,
            tc.tile_pool(name="small", bufs=2) as small,
            tc.tile_pool(name="epool", bufs=3) as epool,
            tc.tile_pool(name="zpool", bufs=4) as zpool,
            tc.tile_pool(name="osb", bufs=4) as osbp,
            tc.tile_pool(name="dram", bufs=1, space="DRAM") as dram,
        ):
            # ---------- loads: ONLY xTs before the stats path ----------
            xTs = consts.tile([128, DT, NS], F32)
            for dt in range(DT):
                nc.sync.dma_start(out=xTs[:, dt, :], in_=xTs_d.ap()[:, dt, :])
            w8 = consts.tile([128, DT, 512], F8)
            xq8t = consts.tile([128, DT, N], F8)
            csw = consts.tile([128, 4], F32)
            bias4 = consts.tile([128, 4], F32)
            mk = consts.tile([128, 2, 8, 128], BF16)
            ob = consts.tile([128, DT], F32)
            b1t = consts.tile([128, FT], F32)
            b2t = consts.tile([128, DT], F32)
            w2ball = consts.tile([128, FT, 2, 128], BF16)

            ident = consts.tile([128, 128], BF16)
            make_identity(nc, ident[:])
            ones_c = consts.tile([128, 1], F32)
            nc.vector.memset(ones_c[:], 1.0)
            ones_r = consts.tile([1, 128], F32)
            nc.vector.memset(ones_r[:], 1.0)
            ones128 = consts.tile([128, 128], BF16)
            nc.vector.memset(ones128[:], 1.0)

            # ---------- state ----------
            qT = state.tile([128, N], BF16)
            kT = state.tile([128, N], BF16)
            vT = state.tile([128, N], BF16)
            gateT = state.tile([128, N], BF16)
            gatedT = state.tile([128, N], BF16)
            vaug = state.tile([128, NT, 130], BF16)
            gfull = state.tile([128, DT, NS], BF16)
            x2T = state.tile([128, DT, NS], F32)
            xn2T = state.tile([128, DT, NS], BF16)
            hT = state.tile([128, FT, NS], BF16, tag="bigshare")
            qkctx = tc.tile_pool(name="qkst", bufs=1)
            qkst = qkctx.__enter__()
            R_bc = qkst.tile([128, N], F32, tag="R_bc")
            G_bc = qkst.tile([128, 4, N], BF16, tag="G_bc")
            S_r = qkst.tile([128, N], BF16, tag="S_r")
            S_m = qkst.tile([128, N], BF16, tag="S_m")

            # ---------- LN1 stats on local f32 shard ----------
            xsum = sqp.tile([128, NS], F32, tag="lnsum")
            nc.vector.tensor_tensor(out=xsum[:], in0=xTs[:, 0, :], in1=xTs[:, 1, :], op=ALU.add)
            for dt in range(2, DT):
                nc.vector.tensor_tensor(out=xsum[:], in0=xsum[:], in1=xTs[:, dt, :], op=ALU.add)
            sqsum = sqp.tile([128, NS], F32, tag="lnsq")
            nc.gpsimd.tensor_tensor(out=sqsum[:], in0=xTs[:, 0, :], in1=xTs[:, 0, :], op=ALU.mult)
            for dt in range(1, DT):
                sq_t = sqp.tile([128, NS], F32, tag="lnsqt")
                eng = nc.vector if dt % 2 else nc.gpsimd
                eng.tensor_tensor(out=sq_t[:], in0=xTs[:, dt, :], in1=xTs[:, dt, :], op=ALU.mult)
                nc.vector.tensor_tensor(out=sqsum[:], in0=sqsum[:], in1=sq_t[:], op=ALU.add)
            with tc.tile_pool(name="lnps", bufs=1, space="PSUM") as lnps:
                ps_mu = lnps.tile([1, NS], F32, tag="mu")
                ps_sq = lnps.tile([1, NS], F32, tag="sq")
                nc.tensor.matmul(ps_mu[:], ones_c[:], xsum[:], start=True, stop=True)
                nc.tensor.matmul(ps_sq[:], ones_c[:], sqsum[:], start=True, stop=True)
                mean_t = small.tile([1, NS], F32)
                nc.vector.tensor_scalar_mul(out=mean_t[:], in0=ps_mu[:], scalar1=1.0 / D)
                ex2_t = small.tile([1, NS], F32)
                nc.vector.tensor_scalar_mul(out=ex2_t[:], in0=ps_sq[:], scalar1=1.0 / D)
            m2_t = small.tile([1, NS], F32)
            nc.vector.tensor_tensor(out=m2_t[:], in0=mean_t[:], in1=mean_t[:], op=ALU.mult)
            var_t = small.tile([1, NS], F32)
            nc.vector.tensor_scalar(out=var_t[:], in0=ex2_t[:], scalar1=float(EPS), scalar2=None,
                                    op0=ALU.add)
            nc.vector.tensor_tensor(out=var_t[:], in0=var_t[:], in1=m2_t[:], op=ALU.subtract)
            rvar_t = small.tile([1, NS], F32)
            nc.vector.reciprocal(out=rvar_t[:], in_=var_t[:])
            rstd_t = small.tile([1, NS], F32)
            nc.scalar.activation(out=rstd_t[:], in_=rvar_t[:], func=AF.Sqrt)
            st_r = small.tile([1, NS], BF16)
            nc.vector.tensor_scalar_mul(out=st_r[:], in0=rstd_t[:], scalar1=1.0 / (SX * SW))
            mneg = small.tile([1, NS], F32)
            nc.vector.tensor_scalar_mul(out=mneg[:], in0=mean_t[:], scalar1=-1.0)
            st_m = small.tile([1, NS], BF16)
            nc.vector.tensor_tensor(out=st_m[:], in0=mneg[:], in1=rstd_t[:], op=ALU.mult)

            nc.vector.memset(S_r[:], 0.0)
            nc.vector.memset(S_m[:], 0.0)

            # ---------- tiny stats AllGather (bf16) ----------
            st_in = dram.tile([2, NS], BF16)
            st_out = dram.tile([NC, 2, NS], BF16)
            nc.gpsimd.dma_start(out=st_in[0:1], in_=st_r[:])
            nc.gpsimd.dma_start(out=st_in[1:2], in_=st_m[:])
            nc.gpsimd.collective_compute(
                "AllGather", ALU.bypass,
                replica_groups=[list(range(NC))],
                ins=[st_in.opt()], outs=[st_out.opt()],
            )
            for c in range(NC):
                for h in range(2):
                    gcol = 128 * (c + 8 * h)
                    nc.sync.dma_start(out=S_r[0:1, gcol:gcol + 128], in_=st_out[c, 0:1, 128 * h:128 * h + 128])
                    nc.sync.dma_start(out=S_m[0:1, gcol:gcol + 128], in_=st_out[c, 1:2, 128 * h:128 * h + 128])

            # ---------- bulk loads (issued after the latency-critical staging) ----------
            for dt in range(0, DT, 2):
                nc.sync.dma_start(out=w8[:, dt:dt + 2, :], in_=w8_d.ap()[:, dt:dt + 2, :])
            for dt in range(DT):
                nc.sync.dma_start(out=xq8t[:, dt, :], in_=xq8_d.ap()[:, dt, :])
            nc.sync.dma_start(out=csw[:], in_=csw_d.ap())
            nc.sync.dma_start(out=bias4[:], in_=bias4_d.ap())
            nc.sync.dma_start(out=mk[:], in_=mk_d.ap())
            nc.sync.dma_start(out=ob[:], in_=ob_d.ap())
            nc.sync.dma_start(out=b1t[:], in_=b1_d.ap())
            nc.sync.dma_start(out=b2t[:], in_=b2_d.ap())
            nc.sync.dma_start(out=w2ball[:], in_=w2_d.ap()[:, :, 6:8, :])

            # ---------- QKV+gate fp8 DoubleRow matmuls over raw x ----------
            with tc.tile_pool(name="qkps", bufs=6, space="PSUM") as qkps:
                ps_list = []
                for g in range(4):
                    for c2 in range(4):
                        ps = qkps.tile([128, 512], F32)
                        for kp in range(4):
                            nc.tensor.matmul(
                                ps[:],
                                w8[:, 2 * kp:2 * kp + 2, 128 * g:128 * g + 128],
                                xq8t[:, 2 * kp:2 * kp + 2, 512 * c2:512 * c2 + 512],
                                start=(kp == 0), stop=(kp == 3),
                                perf_mode=PERF2,
                            )
                        ps_list.append((g, c2, ps))

                # full-rate row broadcasts: all-ones stationary x zeroed row tile
                with tc.tile_pool(name="bcps", bufs=1, space="PSUM") as bcps:
                    for half in range(4):
                        cols = slice(512 * half, 512 * half + 512)
                        psb = bcps.tile([128, 512], F32, tag="rbc")
                        nc.tensor.matmul(psb[:], ones128[:], S_r[:, cols], start=True, stop=True)
                        nc.scalar.activation(out=R_bc[:, cols], in_=psb[:], func=AF.Copy)
                        psm = bcps.tile([128, 512], F32, tag="mbc")
                        nc.tensor.matmul(psm[:], ones128[:], S_m[:, cols], start=True, stop=True)
                        for g in range(4):
                            nc.vector.tensor_scalar(
                                out=G_bc[:, g, cols], in0=psm[:],
                                scalar1=csw[:, g:g + 1], scalar2=bias4[:, g:g + 1],
                                op0=ALU.mult, op1=ALU.add,
                            )

                    # epilogue: dequant + LN fold + bias (+ sigmoid for gate)
                    for g, c2, ps in ps_list:
                        cols = slice(512 * c2, 512 * c2 + 512)
                        t1 = sqp.tile([128, 512], F32, tag="evac")
                        nc.vector.tensor_tensor(out=t1[:], in0=ps[:], in1=R_bc[:, cols], op=ALU.mult)
                        if g < 3:
                            dst = (qT, kT, vT)[g]
                            eng = nc.gpsimd if g in (1, 2) else nc.vector
                            eng.tensor_tensor(out=dst[:, cols], in0=t1[:], in1=G_bc[:, g, cols], op=ALU.add)
                        else:
                            t2 = sqp.tile([128, 512], F32, tag="evac2")
                            nc.vector.tensor_tensor(out=t2[:], in0=t1[:], in1=G_bc[:, g, cols], op=ALU.add)
                            nc.scalar.activation(out=gateT[:, cols], in_=t2[:], func=AF.Sigmoid)

            qkctx.__exit__(None, None, None)

            # ---------- v rows (PE transpose) + ones column ----------
            with tc.tile_pool(name="trps", bufs=2, space="PSUM") as trps:
                for b in range(NT):
                    pst = trps.tile([128, 128], BF16)
                    nc.tensor.transpose(pst[:], vT[:, 128 * b:128 * b + 128], ident[:])
                    nc.scalar.activation(out=vaug[:, b, 0:64], in_=pst[:, 0:64], func=AF.Copy)
                    nc.scalar.activation(out=vaug[:, b, 65:129], in_=pst[:, 64:128], func=AF.Copy)
            nc.vector.memset(vaug[:, :, 64:65], 1.0)
            nc.vector.memset(vaug[:, :, 129:130], 1.0)

            # ---------- attention (token-major, lag-2 deferred transpose) ----------
            a2a_in0 = dram.tile([NC, 128, 128], BF16, tag="a2ain0")
            a2a_in1 = dram.tile([NC, 128, 128], BF16, tag="a2ain1")
            a2a_out0 = dram.tile([NC, 128, 128], BF16, tag="a2aout0")
            a2a_out1 = dram.tile([NC, 128, 128], BF16, tag="a2aout1")
            a2a_in = [a2a_in0, a2a_in1]
            a2a_out = [a2a_out0, a2a_out1]
            with (
                tc.tile_pool(name="scps", bufs=2, space="PSUM") as scps,
                tc.tile_pool(name="ops", bufs=2, space="PSUM") as ops,
                tc.tile_pool(name="tps", bufs=2, space="PSUM") as tps,
            ):
                pending = []

                def emit_tail(t, hp, o_sb):
                    psT = tps.tile([64, 128], BF16)
                    nc.tensor.transpose(psT[:], o_sb[:], ident[:])
                    nc.vector.tensor_tensor(
                        out=gatedT[64 * hp:64 * hp + 64, 128 * t:128 * t + 128],
                        in0=psT[:], in1=gateT[64 * hp:64 * hp + 64, 128 * t:128 * t + 128],
                        op=ALU.mult)

                for ti, t in enumerate(T_ORDER):
                    for hp in range(2):
                        p_t = sum(1 for dl in DELTAS if dl <= t)
                        psS = scps.tile([128, 1024], F32)
                        for s in range(p_t):
                            b = t - DELTAS[s]
                            nc.tensor.matmul(psS[:, 128 * s:128 * s + 128],
                                             kT[64 * hp:64 * hp + 64, 128 * b:128 * b + 128],
                                             qT[64 * hp:64 * hp + 64, 128 * t:128 * t + 128],
                                             start=True, stop=True)
                        E = epool.tile([128, 1024], BF16)
                        nc.scalar.activation(out=E[:, :128 * p_t], in_=psS[:, :128 * p_t],
                                             func=AF.Exp, scale=float(HD ** -0.5))
                        meng = nc.gpsimd if hp == 1 else nc.vector
                        meng.tensor_tensor(out=E[:, :128 * p_t], in0=E[:, :128 * p_t],
                                           in1=mk[:, hp, 0:p_t, :], op=ALU.mult)
                        psO = ops.tile([128, 65], F32)
                        for s in range(p_t):
                            b = t - DELTAS[s]
                            nc.tensor.matmul(psO[:], E[:, 128 * s:128 * s + 128],
                                             vaug[:, b, 65 * hp:65 * hp + 65],
                                             start=(s == 0), stop=(s == p_t - 1))
                        zinv = zpool.tile([128, 1], F32)
                        nc.vector.reciprocal(out=zinv[:], in_=psO[:, 64:65])
                        o_sb = osbp.tile([128, 64], BF16)
                        nc.vector.tensor_scalar_mul(out=o_sb[:], in0=psO[:, 0:64], scalar1=zinv[:, 0:1])
                        pending.append((t, hp, o_sb))
                        while len(pending) > 2:
                            emit_tail(*pending.pop(0))
                    if ti == NT // 2 - 1 or ti == NT - 1:
                        while pending:
                            emit_tail(*pending.pop(0))
                        c = 0 if ti == NT // 2 - 1 else 1
                        nc.gpsimd.dma_start(
                            out=a2a_in[c][:].rearrange("s p n -> p s n"),
                            in_=gatedT[:].rearrange("p (s n) -> p s n", s=NT)[:, 8 * c:8 * c + 8, :])
                        nc.gpsimd.collective_compute(
                            "AllToAll", ALU.bypass,
                            replica_groups=[list(range(NC))],
                            ins=[a2a_in[c].opt()], outs=[a2a_out[c].opt()],
                        )
                        nc.gpsimd.dma_start(out=gfull[:, :, 128 * c:128 * c + 128],
                                            in_=a2a_out[c][:].rearrange("dt p n -> p dt n"))

            w1ctx = tc.tile_pool(name="w1p", bufs=6)
            w1p = w1ctx.__enter__()
            w2ctx = tc.tile_pool(name="w2p", bufs=3)
            w2p = w2ctx.__enter__()

            # ---------- out proj + residual in token halves, LN2 sums inlined ----------
            x2sum = sqp.tile([128, NS], F32, tag="x2sum")
            x2sq = sqp.tile([128, NS], F32, tag="x2sq")
            with tc.tile_pool(name="mps", bufs=2, space="PSUM") as mps:
                for half in range(2):
                    hc = slice(128 * half, 128 * half + 128)
                    for m in range(DT):
                        owt = w1p.tile([128, DT, 128], BF16, tag="wchunk")
                        nc.sync.dma_start(out=owt[:], in_=ow_d.ap()[:, m, :, :])
                        ps = mps.tile([128, 128], F32)
                        for kt in range(DT):
                            nc.tensor.matmul(ps[:], owt[:, kt, :], gfull[:, kt, hc],
                                             start=(kt == 0), stop=(kt == DT - 1))
                        nc.vector.scalar_tensor_tensor(out=x2T[:, m, hc], in0=ps[:],
                                                       scalar=ob[:, m:m + 1], in1=xTs[:, m, hc],
                                                       op0=ALU.add, op1=ALU.add)
                        sqm = sqp.tile([128, 128], F32, tag="lnsqt")
                        nc.vector.tensor_tensor(out=sqm[:], in0=x2T[:, m, hc], in1=x2T[:, m, hc], op=ALU.mult)
                        if m == 0:
                            nc.vector.tensor_copy(out=x2sum[:, hc], in_=x2T[:, m, hc])
                            nc.vector.tensor_copy(out=x2sq[:, hc], in_=sqm[:])
                        else:
                            nc.vector.tensor_tensor(out=x2sum[:, hc], in0=x2sum[:, hc], in1=x2T[:, m, hc], op=ALU.add)
                            nc.vector.tensor_tensor(out=x2sq[:, hc], in0=x2sq[:, hc], in1=sqm[:], op=ALU.add)

            # ---------- LN2 stats + apply ----------
            with tc.tile_pool(name="ln2ps", bufs=1, space="PSUM") as ln2ps:
                ps_mu2 = ln2ps.tile([1, NS], F32, tag="mu2")
                ps_sq2 = ln2ps.tile([1, NS], F32, tag="sq2")
                nc.tensor.matmul(ps_mu2[:], ones_c[:], x2sum[:], start=True, stop=True)
                nc.tensor.matmul(ps_sq2[:], ones_c[:], x2sq[:], start=True, stop=True)
                mean2 = small.tile([1, NS], F32)
                nc.vector.tensor_scalar_mul(out=mean2[:], in0=ps_mu2[:], scalar1=1.0 / D)
                ex22 = small.tile([1, NS], F32)
                nc.vector.tensor_scalar_mul(out=ex22[:], in0=ps_sq2[:], scalar1=1.0 / D)
            m22 = small.tile([1, NS], F32)
            nc.vector.tensor_tensor(out=m22[:], in0=mean2[:], in1=mean2[:], op=ALU.mult)
            var2 = small.tile([1, NS], F32)
            nc.vector.tensor_scalar(out=var2[:], in0=ex22[:], scalar1=float(EPS), scalar2=None,
                                    op0=ALU.add)
            nc.vector.tensor_tensor(out=var2[:], in0=var2[:], in1=m22[:], op=ALU.subtract)
            rvar2 = small.tile([1, NS], F32)
            nc.vector.reciprocal(out=rvar2[:], in_=var2[:])
            rstd2 = small.tile([1, NS], F32)
            nc.scalar.activation(out=rstd2[:], in_=rvar2[:], func=AF.Sqrt)
            with tc.tile_pool(name="lnbc", bufs=1, space="PSUM") as lnbc:
                ps_mbc = lnbc.tile([128, NS], F32, tag="mbc2")
                nc.tensor.matmul(ps_mbc[:], ones_r[:], mean2[:], start=True, stop=True)
                ps_rbc = lnbc.tile([128, NS], F32, tag="rbc2")
                nc.tensor.matmul(ps_rbc[:], ones_r[:], rstd2[:], start=True, stop=True)
                for dt in range(DT):
                    tmp_t = sqp.tile([128, NS], F32, tag="lntmp")
                    nc.vector.tensor_tensor(out=tmp_t[:], in0=x2T[:, dt, :],
                                            in1=ps_mbc[:], op=ALU.subtract)
                    nc.vector.tensor_tensor(out=xn2T[:, dt, :], in0=tmp_t[:],
                                            in1=ps_rbc[:], op=ALU.mult)

            # ---------- FFN: fc1 with 6 fc2 chains interleaved + 2-chain tail ----------
            # each fc2 accumulation chain owns a full PSUM bank (512-col f32
            # pitch): interleaved chains sharing a bank lose data on `start`.
            with tc.tile_pool(name="accps", bufs=1, space="PSUM") as accps:
                psA = accps.tile([128, 6, 512], F32, tag="psA")
                fc2_pend = []

                def emit_fc2A(f, w2t):
                    for m in range(6):
                        nc.tensor.matmul(psA[:, m, 0:NS], w2t[:, m, :], hT[:, f, :],
                                         start=(f == 0), stop=(f == FT - 1))

                with tc.tile_pool(name="f1ps", bufs=2, space="PSUM") as f1ps:
                    for f in range(FT):
                        w1t = w1p.tile([128, DT, 128], BF16, tag="wchunk")
                        nc.sync.dma_start(out=w1t[:], in_=w1_d.ap()[:, f, :, :])
                        ps = f1ps.tile([128, NS], F32)
                        for kt in range(DT):
                            nc.tensor.matmul(ps[:], w1t[:, kt, :], xn2T[:, kt, :],
                                             start=(kt == 0), stop=(kt == DT - 1))
                        nc.scalar.activation(out=hT[:, f, :], in_=ps[:], func=AF.Gelu,
                                             bias=b1t[:, f:f + 1])
                        w2t = w2p.tile([128, 6, 128], BF16, tag="w2a")
                        nc.sync.dma_start(out=w2t[:], in_=w2_d.ap()[:, f, 0:6, :])
                        fc2_pend.append((f, w2t))
                        if len(fc2_pend) > 1:
                            emit_fc2A(*fc2_pend.pop(0))
                    while fc2_pend:
                        emit_fc2A(*fc2_pend.pop(0))
                with tc.tile_pool(name="accpsB", bufs=1, space="PSUM") as accpsB:
                    psB = accpsB.tile([128, 2, 512], F32, tag="psB")
                    for f in range(FT):
                        for m in range(2):
                            nc.tensor.matmul(psB[:, m, 0:NS], w2ball[:, f, m, :], hT[:, f, :],
                                             start=(f == 0), stop=(f == FT - 1))
                    for m in range(DT):
                        src_ps = psA[:, m, 0:NS] if m < 6 else psB[:, m - 6, 0:NS]
                        yt = sqp.tile([128, NS], F32, tag="yout")
                        nc.vector.scalar_tensor_tensor(out=yt[:], in0=src_ps,
                                                       scalar=b2t[:, m:m + 1], in1=x2T[:, m, :],
                                                       op0=ALU.add, op1=ALU.add)
                        nc.sync.dma_start(out=y_d.ap()[:, m, :], in_=yt[:])
            w2ctx.__exit__(None, None, None)
            w1ctx.__exit__(None, None, None)

    nc.finalize()
    return nc


_BF = ml_dtypes.bfloat16
_F8 = mybir.dt.np(F8)


def _bf(a):
    return np.ascontiguousarray(np.asarray(a, dtype=np.float32).astype(_BF))


def _q8(a, scale):
    return np.ascontiguousarray(
        np.clip(np.asarray(a, np.float32) * scale, -240.0, 240.0).astype(_F8))


def _pmajor(w):
    """w (K, M) -> [128, M//128, K//128, 128] p-major contiguous."""
    K, M = w.shape
    return np.ascontiguousarray(
        w.reshape(K // 128, 128, M // 128, 128).transpose(1, 2, 0, 3))


def _ktmajor(w):
    """w (K, M) -> [128, K//128, M//128, 128] kt-major contiguous."""
    K, M = w.shape
    return np.ascontiguousarray(
        w.reshape(K // 128, 128, M // 128, 128).transpose(1, 0, 2, 3))


def _prep_inputs(inputs):
    x = np.asarray(inputs["x"], dtype=np.float32)[0]          # (N, D)
    g1 = np.asarray(inputs["ln1_g"], np.float32); b1 = np.asarray(inputs["ln1_b"], np.float32)
    g2 = np.asarray(inputs["ln2_g"], np.float32); b2 = np.asarray(inputs["ln2_b"], np.float32)
    qkv_w = np.asarray(inputs["qkv_w"], np.float32); qkv_b = np.asarray(inputs["qkv_b"], np.float32)
    gate_w = np.asarray(inputs["gate_w"], np.float32); gate_b = np.asarray(inputs["gate_b"], np.float32)
    out_w = np.asarray(inputs["out_w"], np.float32); out_b = np.asarray(inputs["out_b"], np.float32)
    fc1_w = np.asarray(inputs["fc1_w"], np.float32); fc1_b = np.asarray(inputs["fc1_b"], np.float32)
    fc2_w = np.asarray(inputs["fc2_w"], np.float32); fc2_b = np.asarray(inputs["fc2_b"], np.float32)
    pos_bias = np.asarray(inputs["pos_bias"], np.float32)     # (O, H)

    xT = np.ascontiguousarray(x.T)                            # (D, N)
    qkvw_eff = g1[:, None] * qkv_w
    qkvb_eff = qkv_b + b1 @ qkv_w
    gatew_eff = g1[:, None] * gate_w
    gateb_eff = gate_b + b1 @ gate_w
    fc1w_eff = g2[:, None] * fc1_w
    fc1b_eff = fc1_b + b2 @ fc1_w

    xq8 = np.ascontiguousarray(
        _q8(xT, SX).reshape(DT, 128, N).transpose(1, 0, 2))
    ow_p = _pmajor(_bf(out_w))
    w1_p = _pmajor(_bf(fc1w_eff))
    w2_p = _ktmajor(_bf(fc2_w))
    ob_pack = np.ascontiguousarray(out_b.reshape(DT, 128).T)
    b1_pack = np.ascontiguousarray(fc1b_eff.reshape(FT, 128).T)
    b2_pack = np.ascontiguousarray(fc2_b.reshape(DT, 128).T)

    offs = np.asarray(OFFSETS)
    in_maps = []
    for i in range(NC):
        qcols = slice(128 * i, 128 * i + 128)
        kcols = slice(D + 128 * i, D + 128 * i + 128)
        vcols = slice(2 * D + 128 * i, 2 * D + 128 * i + 128)
        W_all = np.concatenate([qkvw_eff[:, qcols], qkvw_eff[:, kcols],
                                qkvw_eff[:, vcols], gatew_eff[:, qcols]], axis=1)  # (D, 512)
        w8 = np.ascontiguousarray(
            _q8(W_all, SW).reshape(DT, 128, 512).transpose(1, 0, 2))
        csw = np.ascontiguousarray(W_all.sum(0).reshape(4, 128).T.astype(np.float32))
        bias4 = np.stack([qkvb_eff[qcols], qkvb_eff[kcols], qkvb_eff[vcols],
                          gateb_eff[qcols]], axis=1).astype(np.float32)
        masks = np.zeros((2, 8, 128, 128), np.float32)
        jj, nn = np.meshgrid(np.arange(128), np.arange(128), indexing="ij")
        for hp in range(2):
            h = 2 * i + hp
            for s, dl in enumerate(DELTAS):
                delta = 128 * dl + nn - jj
                valid = np.isin(delta, offs)
                pb = np.zeros((128, 128), np.float32)
                pb[valid] = pos_bias[np.searchsorted(offs, delta[valid]), h]
                masks[hp, s] = np.where(valid, np.exp(pb), 0.0)
        mk_p = np.ascontiguousarray(_bf(masks).transpose(2, 0, 1, 3))
        xcols = np.concatenate([xT[:, 128 * i:128 * i + 128],
                                xT[:, 128 * (i + 8):128 * (i + 8) + 128]], axis=1)
        xTs = np.ascontiguousarray(xcols.reshape(DT, 128, NS).transpose(1, 0, 2))
        in_maps.append({
            "xq8": xq8, "xTs": xTs,
            "w8": w8, "csw": csw, "bias4": bias4,
            "out_w": ow_p, "out_b": ob_pack,
            "fc1_w": w1_p, "fc1_b": b1_pack,
            "fc2_w": w2_p, "fc2_b": b2_pack,
            "masks": mk_p,
        })
    return in_maps


_PROGRAM = None


def _get_program():
    global _PROGRAM
    if _PROGRAM is None:
        _PROGRAM = build_program()
    return _PROGRAM


def run(inputs, **run_kwargs):
    nc = _get_program()
    in_maps = _prep_inputs(inputs)
    res = run_bass_kernel_spmd(nc, in_maps, core_ids=list(range(NC)), **run_kwargs)
    yT = np.zeros((D, N), np.float32)
    for i in range(NC):
        yp = res.results[i]["yT"].transpose(1, 0, 2).reshape(D, NS)  # (D, 256)
        yT[:, 128 * i:128 * i + 128] = yp[:, 0:128]
        yT[:, 128 * (i + 8):128 * (i + 8) + 128] = yp[:, 128:256]
    return np.ascontiguousarray(yT.T)[None], res


def kernel(**inputs):
    y, _ = run(inputs)
    return y
